# revision 15
# baseline (speedup 1.0000x reference)
"""Trainium2 Bass kernel for an 8-layer dense transformer (CloudTransformerMM).

Strategy: data-parallel over tokens (zigzag chunk pairing: core c owns chunks
{c, 15-c} of each batch) across 8 cores, per-layer K/V AllGather.
Feature-major residual stream [D, tokens] per core so projections need no
activation transposes.

v2 changes vs baseline:
- All weights stored bf16 in DRAM, pre-transposed into exactly the SBUF tile
  layout (contiguous >=2KB runs per partition) -> half the DMA bytes, no
  strided descriptor hell.
- k-major attention: scores computed as K^T.Q ([tk, tq] layout) so PV needs
  no probs transposes; softmax denominators come free via a ones-column
  appended to V (PV row 64); no max-subtraction (scores empirically < ~20);
  normalization fused post-PV via a 1-row broadcast matmul.
- V AllGather in bf16 (half bytes).
Precision: bf16 weights x f32r activations on PE (full speed, moving operand
is f32r with N=512), bf16 score path, fp32 PSUM + residual.
"""
import math
import sys

sys.path.insert(0, '/opt/trn_rl_repo')

import numpy as np
import ml_dtypes

B, S, D = 2, 2048, 1024
NH, KVH, HD = 16, 4, 64
L, DFF, V = 8, 4096, 32000
THETA, YSCALE, YALPHA, YBETA = 10000.0, 40.0, 1.0, 32.0
ROPE_MAX = 2048 * 40
EPS = 1e-6

NC = 8
NCH = 16
CH = S // NCH          # 128
TPC = 2 * 2 * CH       # 512
KS = D // 128          # 8
W0, W1 = 1024, 2048    # score window widths (incl. own slot) for qi = 0 / 1
NEG = -1.0e30
NVCH = (V + 511) // 512  # 63 (last chunk zero-padded host-side)

bf16 = ml_dtypes.bfloat16


def rope_tables():
    inv_freq = 1.0 / THETA ** (np.arange(0, HD, 2, dtype=np.float32) / HD)
    wavelengths = 2.0 * math.pi / inv_freq
    r = ROPE_MAX / wavelengths
    gamma = np.clip((r - YALPHA) / (YBETA - YALPHA), 0.0, 1.0)
    inv_freq = inv_freq * ((1.0 - gamma) / YSCALE + gamma)
    t = np.arange(S, dtype=np.float32)
    freqs = np.outer(t, inv_freq)
    emb = np.concatenate([freqs, freqs], axis=-1)
    emb = emb / math.sqrt(0.1 * math.log(YSCALE) + 1.0)
    return np.cos(emb).astype(np.float32), np.sin(emb).astype(np.float32)


def core_chunks(c):
    return [c, NCH - 1 - c]


def chunk_owner(lk):
    oc = min(lk, NCH - 1 - lk)
    return oc, (0 if lk == oc else 1)


_NC_CACHE = None


def build_nc():
    global _NC_CACHE
    if _NC_CACHE is not None:
        return _NC_CACHE
    import concourse.mybir as mybir
    import concourse.tile as tile
    from concourse import bacc

    f32 = mybir.dt.float32
    f32r = mybir.dt.float32r
    bfl = mybir.dt.bfloat16
    AF = mybir.ActivationFunctionType
    ALU = mybir.AluOpType

    nc = bacc.Bacc("TRN2", target_bir_lowering=False, debug=False,
                   enable_asserts=True, num_devices=NC)

    def din(name, shape, dt):
        return nc.dram_tensor(name, shape, dt, kind="ExternalInput").ap()

    x0T_d = din("x0T", [128, KS, TPC], f32)
    wq_d = din("wqT", [L * KS, 128, KS, 128], bfl)     # [l*8+mb]
    wk_d = din("wkT", [L, 128, KS, KVH * HD], bfl)
    wv_d = din("wvT", [L, 128, KS, KVH * HD], bfl)
    wo_d = din("woT", [L * KS, 128, KS, 128], bfl)     # [l*8+mb]
    w1_d = din("w1T", [L * 8, 128, KS, 512], bfl)      # [l*8+q8]
    w3_d = din("w3T", [L * 8, 128, KS, 512], bfl)
    w2_d = din("w2T", [L * 4 * KS, 128, 8, 128], bfl)  # [(l*4+qu)*8+mb]
    n1_d = din("n1", [L, 128, KS], f32)
    n2_d = din("n2", [L, 128, KS], f32)
    fnw_d = din("fnw", [128, KS], f32)
    emb_d = din("embT", [NVCH, 128, KS, 512], bfl)
    cosq_d = din("cosq", [HD, TPC], f32)
    sinq_d = din("sinq", [HD, TPC], f32)
    cosk_d = din("cosk", [HD, TPC], f32)
    sink_d = din("sink", [HD, TPC], f32)
    p64_d = din("p64", [HD, HD], f32r)
    trilT_d = din("trilT", [128, 128], f32)
    ones_d = din("ones128", [128, 128], f32r)
    kbias_d = din("kbias", [2, KVH, W1], bfl)
    qflag_d = din("qflag", [2, NH, TPC], bfl)
    out_d = nc.dram_tensor("out", [TPC, V], f32, kind="ExternalOutput").ap()

    with tile.TileContext(nc) as tc, \
         tc.tile_pool(name="pers", bufs=1) as pers:
        hT = pers.tile([128, KS, TPC], f32, tag="hT", name="hT")
        qrot = pers.tile([128, NH, TPC], bfl, tag="qrot", name="qrot")
        cosq = pers.tile([HD, TPC], f32, tag="cosq", name="cosq")
        sinq = pers.tile([HD, TPC], f32, tag="sinq", name="sinq")
        cosk = pers.tile([HD, TPC], f32, tag="cosk", name="cosk")
        sink = pers.tile([HD, TPC], f32, tag="sink", name="sink")
        p64 = pers.tile([HD, HD], f32r, tag="p64", name="p64")
        trilT = pers.tile([128, 128], f32, tag="trilT", name="trilT")
        ones128 = pers.tile([128, 128], f32r, tag="ones128", name="ones128")

        nc.sync.dma_start(hT[:], x0T_d[:])
        nc.sync.dma_start(qrot[64:66, :, :], qflag_d[:])
        for t_, d_ in ((cosq, cosq_d), (sinq, sinq_d), (cosk, cosk_d),
                       (sink, sink_d), (p64, p64_d), (trilT, trilT_d),
                       (ones128, ones_d)):
            nc.sync.dma_start(t_[:], d_[:])

        def rmsnorm(P, smp, src, w_sb, dst):
            ssp = P.tile([128, TPC], f32, tag="mm", name="ssp")
            for sub in range(KS):
                sq = smp.tile([128, TPC], f32r, tag="sq", name="sq")
                nc.scalar.activation(sq[:], src[:, sub, :], AF.Square)
                nc.tensor.matmul(ssp[:], ones128[:], sq[:],
                                 start=(sub == 0), stop=(sub == KS - 1))
            sd2 = smp.tile([128, TPC], f32, tag="sd2", name="sd2")
            nc.vector.tensor_scalar(sd2[:], ssp[:], 1.0 / D, float(EPS),
                                    ALU.mult, ALU.add)
            sd = smp.tile([128, TPC], f32, tag="sd", name="sd")
            nc.scalar.activation(sd[:], sd2[:], AF.Sqrt)
            inv = smp.tile([128, TPC], f32, tag="inv", name="inv")
            nc.vector.reciprocal(inv[:], sd[:])
            for sub in range(KS):
                nc.vector.scalar_tensor_tensor(
                    dst[:, sub, :], src[:, sub, :], w_sb[:, sub:sub + 1],
                    inv[:], ALU.mult, ALU.mult)

        with tc.tile_pool(name="P", bufs=2, space="PSUM") as P, \
             tc.tile_pool(name="dram", bufs=2, space="DRAM") as dram:

            for l in range(L):
                with tc.tile_pool(name="layerp", bufs=1) as lp:
                    xn = lp.tile([128, KS, TPC], bfl, tag="xn", name="xn")
                    kr = lp.tile([64, KVH, TPC], bfl, tag="kr", name="kr")
                    v_s = lp.tile([128, 4, KVH, HD + 1], bfl, tag="v_s",
                                  name="v_s")
                    o_sb = lp.tile([128, KS, TPC], bfl, tag="o", name="o_sb")

                    # ======== phase A: norm1, k/v/q proj + rope + gathers ====
                    with tc.tile_pool(name="phA", bufs=2) as pa, \
                         tc.tile_pool(name="PpsA", bufs=4,
                                      space="PSUM") as ppa:
                        n1sb = pa.tile([128, KS], f32, tag="nw", name="n1sb")
                        nc.sync.dma_start(n1sb[:], n1_d[l])
                        rmsnorm(P, pa, hT, n1sb, xn)

                        # k projection + rope + gather (first, to hide latency)
                        wk_t = pa.tile([128, KS, KVH * HD], bfl, tag="wkv",
                                       name="wk_t")
                        nc.sync.dma_start(wk_t[:], wk_d[l])
                        k_s = pa.tile([64, KVH, TPC], f32r, tag="k_s", bufs=1,
                                      name="k_s")
                        for mb in range(2):
                            pk = ppa.tile([128, TPC], f32, tag="mm", name="pk")
                            for k in range(KS):
                                nc.tensor.matmul(
                                    pk[:], wk_t[:, k, mb * 128:(mb + 1) * 128],
                                    xn[:, k, :], start=(k == 0),
                                    stop=(k == KS - 1))
                            nc.scalar.copy(k_s[:, 2 * mb, :], pk[0:64, :])
                            nc.scalar.copy(k_s[:, 2 * mb + 1, :], pk[64:128, :])
                        for g in range(KVH):
                            psh = ppa.tile([64, TPC], f32, tag="mm", name="psh")
                            nc.tensor.matmul(psh[:], p64[:], k_s[:, g, :],
                                             start=True, stop=True)
                            tA = pa.tile([64, TPC], f32, tag="tA", name="tA")
                            nc.vector.tensor_mul(tA[:], psh[:], sink[:])
                            tB = pa.tile([64, TPC], f32, tag="tB", name="tB")
                            nc.vector.tensor_mul(tB[:], k_s[:, g, :], cosk[:])
                            nc.vector.tensor_add(kr[:, g, :], tA[:], tB[:])
                        kga_in = dram.tile([64, KVH, TPC], bfl, tag="kga_i",
                                           name="kga_in")
                        nc.sync.dma_start(kga_in[:], kr[:])
                        kga_out = dram.tile([NC, 64, KVH, TPC], bfl, tag="kga_o",
                                            addr_space="Shared", name="kga_out")
                        nc.gpsimd.collective_compute(
                            "AllGather", ALU.bypass,
                            replica_groups=[list(range(NC))],
                            ins=[kga_in.opt()], outs=[kga_out.opt()])

                        # v projection (token-major, bf16, +ones col) + gather
                        wv_t = pa.tile([128, KS, KVH * HD], bfl, tag="wkv",
                                       name="wv_t")
                        nc.sync.dma_start(wv_t[:], wv_d[l])
                        nc.vector.memset(v_s[:, :, :, HD:HD + 1], 1.0)
                        for tb in range(4):
                            pv_ = ppa.tile([128, KVH * HD], f32, tag="mm",
                                         name="pv_")
                            for k in range(KS):
                                nc.tensor.matmul(
                                    pv_[:], xn[:, k, tb * 128:(tb + 1) * 128],
                                    wv_t[:, k, :], start=(k == 0),
                                    stop=(k == KS - 1))
                            nc.scalar.copy(
                                v_s[:, tb, :, 0:HD],
                                pv_[:].rearrange("p (g h) -> p g h", g=KVH))
                        vga_in = dram.tile([128, 4, KVH * HD], bfl, tag="vga_i",
                                           name="vga_in")
                        nc.sync.dma_start(
                            vga_in[:].rearrange("p t (g h) -> p t g h", g=KVH),
                            v_s[:, :, :, 0:HD])
                        vga_out = dram.tile([NC, 128, 4, KVH * HD], bfl,
                                            tag="vga_o", addr_space="Shared",
                                            name="vga_out")
                        nc.gpsimd.collective_compute(
                            "AllGather", ALU.bypass,
                            replica_groups=[list(range(NC))],
                            ins=[vga_in.opt()], outs=[vga_out.opt()])

                        # q projection + rope
                        for mb in range(KS):
                            wq_t = pa.tile([128, KS, 128], bfl, tag="wqo",
                                           name="wq_t")
                            nc.sync.dma_start(wq_t[:], wq_d[l * KS + mb])
                            pq = ppa.tile([128, TPC], f32, tag="mm", name="pq")
                            for k in range(KS):
                                nc.tensor.matmul(pq[:], wq_t[:, k, :],
                                                 xn[:, k, :], start=(k == 0),
                                                 stop=(k == KS - 1))
                            q_s = pa.tile([64, 2, TPC], f32r, tag="q_s",
                                          name="q_s")
                            nc.scalar.copy(q_s[:, 0, :], pq[0:64, :])
                            nc.scalar.copy(q_s[:, 1, :], pq[64:128, :])
                            for hh in range(2):
                                h_ = 2 * mb + hh
                                psh = ppa.tile([64, TPC], f32, tag="mm",
                                             name="pshq")
                                nc.tensor.matmul(psh[:], p64[:], q_s[:, hh, :],
                                                 start=True, stop=True)
                                tA = pa.tile([64, TPC], f32, tag="tA",
                                             name="tAq")
                                nc.vector.tensor_mul(tA[:], psh[:], sinq[:])
                                tB = pa.tile([64, TPC], f32, tag="tB",
                                             name="tBq")
                                nc.vector.tensor_mul(tB[:], q_s[:, hh, :],
                                                     cosq[:])
                                nc.vector.tensor_add(qrot[0:64, h_, :],
                                                     tA[:], tB[:])

                    # ======== phase B: attention (k-major) ===================
                    with tc.tile_pool(name="phB", bufs=2) as pb, \
                         tc.tile_pool(name="Psc", bufs=3, space="PSUM") as Psc, \
                         tc.tile_pool(name="Ppv", bufs=2, space="PSUM") as Ppv, \
                         tc.tile_pool(name="Pbc", bufs=1, space="PSUM") as Pbc:
                        for b in range(2):
                            Kg = pb.tile([128, KVH, W1], bfl, tag="Kg", bufs=2,
                                         name="Kg")
                            Vg = pb.tile([128, NCH, KVH, HD + 1], bfl,
                                         tag="Vg", bufs=2, name="Vg")
                            for lk in range(NCH):
                                oc, slot = chunk_owner(lk)
                                blk = 2 * b + slot
                                nc.sync.dma_start(
                                    Kg[0:64, :, lk * 128:(lk + 1) * 128],
                                    kga_out[oc, :, :, blk * 128:(blk + 1) * 128])
                                nc.sync.dma_start(
                                    Vg[:, lk, :, 0:HD],
                                    vga_out[oc, :, blk, :]
                                    .rearrange("p (g h) -> p g h", g=KVH))
                            nc.vector.memset(Vg[:, :, :, HD:HD + 1], 1.0)
                            nc.sync.dma_start(Kg[64:66, :, :], kbias_d[:])

                            for qi in range(2):
                                qb = 2 * b + qi
                                qs = slice(qb * 128, (qb + 1) * 128)
                                nwin = (W0 if qi == 0 else W1) // 128 - 1
                                for g in range(KVH):
                                    pvq = Ppv.tile([HD + 1, 4 * 128], f32,
                                                   tag="pvq", name="pvq")
                                    for sl in range(nwin + 1):
                                        sc = Psc.tile([128, 4 * 128], f32,
                                                      tag="sc", name="sc")
                                        if sl < nwin:
                                            nc.tensor.matmul(
                                                sc[:], Kg[0:66, g,
                                                          sl * 128:(sl + 1) * 128],
                                                qrot[0:66, 4 * g:4 * g + 4, qs],
                                                start=True, stop=True)
                                        else:
                                            nc.tensor.matmul(
                                                sc[:], kr[:, g, qs],
                                                qrot[0:64, 4 * g:4 * g + 4, qs],
                                                start=True, stop=True)
                                            nc.vector.tensor_add(
                                                sc[:].rearrange(
                                                    "p (h q) -> p h q", h=4),
                                                sc[:].rearrange(
                                                    "p (h q) -> p h q", h=4),
                                                trilT[:, None, :]
                                                .to_broadcast([128, 4, 128]))
                                        probs = pb.tile([128, 4 * 128], bfl,
                                                        tag="probs", bufs=4,
                                                        name="probs")
                                        nc.scalar.activation(probs[:], sc[:],
                                                             AF.Exp)
                                        vsl = (Vg[:, sl, g, :] if sl < nwin
                                               else v_s[:, qb, g, :])
                                        nc.tensor.matmul(
                                            pvq[:], vsl, probs[:],
                                            start=(sl == 0),
                                            stop=(sl == nwin))
                                    # normalize: rec of denom row, broadcast,
                                    # fused copy into o_sb
                                    rec = pb.tile([1, 4 * 128], f32r,
                                                  tag="rec", name="rec")
                                    with nc.allow_low_precision(
                                            reason="f32r recip for PV scale"):
                                        nc.vector.reciprocal(
                                            rec[:], pvq[HD:HD + 1, :])
                                    bc = Pbc.tile([64, 4 * 128], f32,
                                                  tag="bc", name="bc")
                                    nc.tensor.matmul(
                                        bc[:], ones128[0:1, 0:64], rec[:],
                                        start=True, stop=True)
                                    bc_sb = pb.tile([64, 4 * 128], f32,
                                                    tag="bcs", name="bc_sb")
                                    nc.vector.tensor_copy(bc_sb[:], bc[:])
                                    for hh in range(4):
                                        h_ = 4 * g + hh
                                        nc.vector.tensor_mul(
                                            o_sb[64 * (hh % 2):
                                                 64 * (hh % 2) + 64,
                                                 h_ // 2, qs],
                                            pvq[0:HD,
                                                hh * 128:(hh + 1) * 128],
                                            bc_sb[:, hh * 128:(hh + 1) * 128])

                        # wo projection + residual
                        for mb in range(KS):
                            wo_t = pb.tile([128, KS, 128], bfl, tag="wqo",
                                           name="wo_t")
                            nc.sync.dma_start(wo_t[:], wo_d[l * KS + mb])
                            po = P.tile([128, TPC], f32, tag="mm", name="po")
                            for k in range(KS):
                                nc.tensor.matmul(po[:], wo_t[:, k, :],
                                                 o_sb[:, k, :], start=(k == 0),
                                                 stop=(k == KS - 1))
                            nc.vector.tensor_add(hT[:, mb, :], hT[:, mb, :],
                                                 po[:])

                    # ======== phase C: mlp ===================================
                    with tc.tile_pool(name="phC", bufs=2) as pc_, \
                         tc.tile_pool(name="Pmlp", bufs=6, space="PSUM") as pml:
                        n2sb = pc_.tile([128, KS], f32, tag="nw", name="n2sb")
                        nc.sync.dma_start(n2sb[:], n2_d[l])
                        y = lp.tile([128, KS, TPC], bfl, tag="y", name="y")
                        rmsnorm(P, pc_, hT, n2sb, y)
                        for quarter in range(4):
                            m_sb = pc_.tile([128, 8, TPC], bfl, tag="m",
                                            bufs=1, name="m_sb")
                            for mb4 in range(2):
                                q8 = quarter * 2 + mb4
                                w1_t = pc_.tile([128, KS, 512], bfl, tag="w13",
                                                name="w1_t")
                                nc.sync.dma_start(w1_t[:], w1_d[l * 8 + q8])
                                w3_t = pc_.tile([128, KS, 512], bfl, tag="w13",
                                                name="w3_t")
                                nc.sync.dma_start(w3_t[:], w3_d[l * 8 + q8])
                                for mbi in range(4):
                                    pu = pml.tile([128, TPC], f32, tag="mm",
                                                name="pu")
                                    for k in range(KS):
                                        nc.tensor.matmul(
                                            pu[:],
                                            w1_t[:, k, mbi * 128:(mbi + 1) * 128],
                                            y[:, k, :], start=(k == 0),
                                            stop=(k == KS - 1))
                                    s_sb = pc_.tile([128, TPC], f32r, tag="s",
                                                    name="s_sb")
                                    nc.scalar.activation(s_sb[:], pu[:], AF.Silu)
                                    pg = pml.tile([128, TPC], f32, tag="mm",
                                                name="pg")
                                    for k in range(KS):
                                        nc.tensor.matmul(
                                            pg[:],
                                            w3_t[:, k, mbi * 128:(mbi + 1) * 128],
                                            y[:, k, :], start=(k == 0),
                                            stop=(k == KS - 1))
                                    nc.vector.tensor_mul(
                                        m_sb[:, mb4 * 4 + mbi, :], s_sb[:],
                                        pg[:])
                            for mb in range(KS):
                                w2_t = pc_.tile([128, 8, 128], bfl, tag="w2",
                                                name="w2_t")
                                nc.sync.dma_start(
                                    w2_t[:], w2_d[(l * 4 + quarter) * KS + mb])
                                pd = pml.tile([128, TPC], f32, tag="mm", name="pd")
                                for ks_ in range(8):
                                    nc.tensor.matmul(pd[:], w2_t[:, ks_, :],
                                                     m_sb[:, ks_, :],
                                                     start=(ks_ == 0),
                                                     stop=(ks_ == 7))
                                nc.vector.tensor_add(hT[:, mb, :], hT[:, mb, :],
                                                     pd[:])

            # ======== final norm + lm head ===================================
            with tc.tile_pool(name="phL", bufs=2) as pl_, \
                 tc.tile_pool(name="Plm", bufs=6, space="PSUM") as plp:
                fnsb = pl_.tile([128, KS], f32, tag="nw", name="fnsb")
                nc.sync.dma_start(fnsb[:], fnw_d[:])
                hn = pl_.tile([128, KS, TPC], bfl, tag="hn", bufs=1, name="hn")
                rmsnorm(P, pl_, hT, fnsb, hn)
                for vch in range(NVCH):
                    n = min(512, V - vch * 512)
                    emb_t = pl_.tile([128, KS, 512], bfl, tag="emb",
                                     name="emb_t")
                    nc.sync.dma_start(emb_t[:], emb_d[vch])
                    for tb in range(4):
                        plm = plp.tile([128, 512], f32, tag="mm", name="plm")
                        for k in range(KS):
                            nc.tensor.matmul(
                                plm[:, 0:n],
                                hn[:, k, tb * 128:(tb + 1) * 128],
                                emb_t[:, k, 0:n], start=(k == 0),
                                stop=(k == KS - 1))
                        ol = pl_.tile([128, 512], f32, tag="ol", name="ol")
                        nc.scalar.copy(ol[:, 0:n], plm[:, 0:n])
                        nc.sync.dma_start(
                            out_d[tb * 128:(tb + 1) * 128,
                                  vch * 512:vch * 512 + n], ol[:, 0:n])
    nc.compile()
    _NC_CACHE = nc
    return nc


def host_prep(inputs):
    """Build per-core in_maps. Weights are pre-transposed host-side into the
    exact SBUF tile layouts (contiguous DMA runs) and cast to bf16."""
    ids = np.asarray(inputs['input_ids'])
    emb = np.asarray(inputs['tok_embed'], np.float32)
    wq = np.asarray(inputs['wq'], np.float32)
    wk = np.asarray(inputs['wk'], np.float32)
    wv = np.asarray(inputs['wv'], np.float32)
    wo = np.asarray(inputs['wo'], np.float32)
    n1 = np.asarray(inputs['norm1_w'], np.float32)
    n2 = np.asarray(inputs['norm2_w'], np.float32)
    w1 = np.asarray(inputs['w1'], np.float32)
    w2 = np.asarray(inputs['w2'], np.float32)
    w3 = np.asarray(inputs['w3'], np.float32)
    fnw = np.asarray(inputs['final_norm_w'], np.float32)

    cos, sin = rope_tables()
    scale = np.float32(HD ** -0.5)
    sgn = np.concatenate([-np.ones(HD // 2, np.float32),
                          np.ones(HD // 2, np.float32)])

    # weight layouts: target[l, mb, p, k, mm] = w[l, mb*128+mm, k*128+p]
    wqT = np.ascontiguousarray(
        wq.reshape(L, KS, 128, KS, 128).transpose(0, 1, 4, 3, 2)
    ).reshape(L * KS, 128, KS, 128).astype(bf16)
    woT = np.ascontiguousarray(
        wo.reshape(L, KS, 128, KS, 128).transpose(0, 1, 4, 3, 2)
    ).reshape(L * KS, 128, KS, 128).astype(bf16)
    # [l, p, k, m] = w[l, m, k*128+p], m in 0..255
    wkT = np.ascontiguousarray(
        wk.reshape(L, 256, KS, 128).transpose(0, 3, 2, 1)).astype(bf16)
    wvT = np.ascontiguousarray(
        wv.reshape(L, 256, KS, 128).transpose(0, 3, 2, 1)).astype(bf16)
    # [l, q8, p, k, mm(512)] = w[l, q8*512+mm, k*128+p]
    w1T = np.ascontiguousarray(
        w1.reshape(L, 8, 512, KS, 128).transpose(0, 1, 4, 3, 2)
    ).reshape(L * 8, 128, KS, 512).astype(bf16)
    w3T = np.ascontiguousarray(
        w3.reshape(L, 8, 512, KS, 128).transpose(0, 1, 4, 3, 2)
    ).reshape(L * 8, 128, KS, 512).astype(bf16)
    # [l, qu, mb, p, ks, mm] = w2[l, mb*128+mm, qu*1024+ks*128+p]
    w2T = np.ascontiguousarray(
        w2.reshape(L, KS, 128, 4, 8, 128).transpose(0, 3, 1, 5, 4, 2)
    ).reshape(L * 4 * KS, 128, 8, 128).astype(bf16)
    # [vch, p, k, vv] = emb[vch*512+vv, k*128+p]
    embp = np.zeros((NVCH * 512, D), np.float32)
    embp[0:V] = emb
    embT = np.ascontiguousarray(
        embp.reshape(NVCH, 512, KS, 128).transpose(0, 3, 2, 1)).astype(bf16)

    shared = {
        "wqT": wqT, "woT": woT, "wkT": wkT, "wvT": wvT,
        "w1T": w1T, "w3T": w3T, "w2T": w2T, "embT": embT,
        "n1": np.ascontiguousarray(n1.reshape(L, KS, 128).transpose(0, 2, 1)),
        "n2": np.ascontiguousarray(n2.reshape(L, KS, 128).transpose(0, 2, 1)),
        "fnw": np.ascontiguousarray(fnw.reshape(KS, 128).T),
        "p64": np.eye(HD, dtype=np.float32)[
            np.concatenate([np.arange(32, 64), np.arange(0, 32)])].T.copy(),
        "ones128": np.ones((128, 128), np.float32),
        # [tk, tq] orientation: invalid where tk > tq
        "trilT": np.tril(np.full((128, 128), NEG, np.float32), -1),
    }
    # qflag rows: row0 selects qi=0 blocks (qb 0,2), row1 selects qi=1
    qf = np.zeros((2, NH, TPC), np.float32)
    for qb in range(4):
        qf[qb % 2, :, qb * 128:(qb + 1) * 128] = 1.0
    shared["qflag"] = qf.astype(bf16)

    in_maps = []
    for c in range(NC):
        pos = []
        for b in range(B):
            for j in core_chunks(c):
                pos.extend((b, j * CH + i) for i in range(CH))
        bidx = np.array([p[0] for p in pos])
        pidx = np.array([p[1] for p in pos])
        x0 = emb[ids[bidx, pidx]]                    # [512, D]
        # x0T[p, k, t] = x0[t, k*128+p]
        x0T = np.ascontiguousarray(
            x0.reshape(TPC, KS, 128).transpose(2, 1, 0))
        cq = np.ascontiguousarray(cos[pidx].T) * scale
        sq = np.ascontiguousarray(sin[pidx].T) * sgn[:, None] * scale
        ck = np.ascontiguousarray(cos[pidx].T)
        sk = np.ascontiguousarray(sin[pidx].T) * sgn[:, None]
        # kbias rows: [qi, g, key] — window chunk lk valid iff lk < own chunk j
        kb = np.zeros((2, KVH, W1), np.float32)
        for qi, j in enumerate(core_chunks(c)):
            nwin = (W0 if qi == 0 else W1) // 128 - 1
            for lk in range(NCH - 1):
                val = 0.0 if lk < j else NEG
                if lk < nwin:
                    kb[qi, :, lk * 128:(lk + 1) * 128] = val
        m = {"x0T": x0T, "cosq": cq.astype(np.float32),
             "sinq": sq.astype(np.float32), "cosk": ck.astype(np.float32),
             "sink": sk.astype(np.float32),
             "kbias": kb.astype(bf16)}
        m.update(shared)
        in_maps.append(m)
    return in_maps


def unshard(results):
    out = np.zeros((B, S, V), np.float32)
    for c in range(NC):
        logits = results[c]["out"]
        for b in range(B):
            for qi, j in enumerate(core_chunks(c)):
                qb = 2 * b + qi
                out[b, j * CH:(j + 1) * CH] = logits[qb * 128:(qb + 1) * 128]
    return out


def kernel(**inputs) -> np.ndarray:
    from concourse.bass_utils import run_bass_kernel_spmd
    nc = build_nc()
    in_maps = host_prep(inputs)
    res = run_bass_kernel_spmd(nc, in_maps, core_ids=list(range(NC)),
                               trace=False)
    return unshard(res.results)
